# revision 33
# baseline (speedup 1.0000x reference)
"""Trainium2 Bass kernel for nn_CrossAttention (b=2, n=m=2048, dim=1024, 16 heads x 64).

Sharding: 8 cores = (batch b in {0,1}) x (head-group g in {0..3}, 4 heads each).
Per core: project q/k/v for its 4 heads (feature-major layouts), attention with
softmax (no max subtraction -- logits are bounded ~|2.7|), row sums via a ones
column appended to V, then an AllToAll over the 4 cores of each batch converts
head-sharding to row-sharding for the output MLP (relu(A@W1)@W2) + LayerNorm.

v3: chunk-batched input DMAs (few dispatches, streaming-ordered), PSUM->SBUF
copies on DVE/Pool only (ACT runs exp exclusively -> single act-table load),
attn@V emission skewed one j-tile behind scores to hide the softmax round trip,
leftover Q projection interleaved into the ACT-bound attention phase, a2a
staging DMAs on the Pool queue overlapped with attention, LN rsqrt via
exp(-0.5*ln(x)) to avoid an activation-table swap.
"""

import sys

if "/opt/trn_rl_repo" not in sys.path:
    sys.path.insert(0, "/opt/trn_rl_repo")

from contextlib import ExitStack

import numpy as np
import ml_dtypes

import concourse.bacc as bacc
import concourse.tile as tile
from concourse import mybir, library_config
from concourse.bass_utils import run_bass_kernel_spmd

DT = mybir.dt
BF16 = ml_dtypes.bfloat16

P = 128          # partitions
N = 2048         # tokens per batch
DIM = 1024       # model dim
HD = 64          # head dim
NH = 4           # heads per core
E = NH * HD      # 256 features per core
CT = DIM // P    # 8 contraction tiles
JT = N // P      # 16 key tiles
IBS = 512        # i-block size
IB = N // IBS    # 4 i-blocks
RQ = 512         # output rows per core
FT = DIM // P    # 8 f-tiles in MLP

_PROGRAM = None
LAST_RUN = None  # BassKernelResults of the most recent kernel() call

import os as _os
SPLIT_A2A = _os.environ.get("SPLIT_A2A", "1") == "1"
STAGE_POOL = _os.environ.get("STAGE_POOL", "1") == "1"
W_ACT = _os.environ.get("W_ACT", "1") == "1"
# ACT-queue DMAs inside a tc.If branch fail at runtime; keep reads on SP.
READ_ACT = _os.environ.get("READ_ACT", "0") == "1"


def _emit_body(nc, tc, tensors, a2a_bufs, a2a_local=False):
    (xT_d, ctxT_d, wq_d, wk_d, wv_d, w1_d, w2_d, gamma_d, out_d) = tensors
    stack = ExitStack()
    with stack:
        const = stack.enter_context(tc.tile_pool(name="const", bufs=1))
        eps_t = const.tile([P, 1], DT.float32, name="eps_t", tag="eps_t")
        nc.vector.memset(eps_t[:], 1e-5)

        # warm the exp activation table immediately (off the critical path)
        warm = const.tile([P, 1], DT.float32, name="warm", tag="warm")
        nc.scalar.activation(warm[:], eps_t[:], mybir.ActivationFunctionType.Exp)

        # ---- persistent activation tiles ----
        qkv = stack.enter_context(tc.tile_pool(name="qkv", bufs=1))
        qT_t = [qkv.tile([P, N], DT.bfloat16, name=f"qT{i}", tag=f"qT{i}") for i in range(2)]
        kT_t = [qkv.tile([P, N], DT.bfloat16, name=f"kT{i}", tag=f"kT{i}") for i in range(2)]
        v_t = [qkv.tile([P, NH * 65], DT.bfloat16, name=f"v{j}", tag=f"v{j}") for j in range(JT)]
        aT_t = [qkv.tile([P, N], DT.bfloat16, name=f"aT{i}", tag=f"aT{i}") for i in range(2)]

        # ---- chunk-batched input loads on the SP queue, in consumption order
        inp = stack.enter_context(tc.tile_pool(name="inputs", bufs=1))
        ctx_a = inp.tile([P, CT, N], DT.bfloat16, name="ctx_a", tag="ctx_a")
        x_a = inp.tile([P, CT, N], DT.bfloat16, name="x_a", tag="x_a")
        wq_a = inp.tile([P, CT, E], DT.bfloat16, name="wq_a", tag="wq_a")
        wk_a = inp.tile([P, CT, E], DT.bfloat16, name="wk_a", tag="wk_a")
        wv_a = inp.tile([P, CT, E], DT.bfloat16, name="wv_a", tag="wv_a")
        # small weights on the ACT queue so they don't delay ctx on SP
        _weng = nc.scalar if W_ACT else nc.sync
        _weng.dma_start(wk_a[:], wk_d[:, :].rearrange("(c p) e -> p c e", c=CT))
        _weng.dma_start(wv_a[:], wv_d[:, :].rearrange("(c p) e -> p c e", c=CT))
        _weng.dma_start(wq_a[:], wq_d[:, :].rearrange("(c p) e -> p c e", c=CT))
        for h in range(4):       # ctx in 4 chunks of 2 c-tiles
            nc.sync.dma_start(
                ctx_a[:, 2 * h:2 * h + 2, :],
                ctxT_d[256 * h:256 * (h + 1), :].rearrange("(c p) n -> p c n", c=2))
        for h in range(2):       # x in 2 chunks of 4 c-tiles
            nc.sync.dma_start(
                x_a[:, 4 * h:4 * h + 4, :],
                xT_d[512 * h:512 * (h + 1), :].rearrange("(c p) n -> p c n", c=4))
        gamma_sb = const.tile([1, DIM], DT.float32, name="gamma_sb", tag="gamma_sb")
        nc.sync.dma_start(gamma_sb[:], gamma_d[:])
        gamma_bc = const.tile([P, DIM], DT.float32, name="gamma_bc", tag="gamma_bc")
        nc.gpsimd.partition_broadcast(gamma_bc[:], gamma_sb[:])
        # MLP weights land while attention runs
        mlpw = stack.enter_context(tc.tile_pool(name="mlpw", bufs=1))
        w1_a = mlpw.tile([P, CT, DIM], DT.bfloat16, name="w1_a", tag="w1_a")
        w2_a = mlpw.tile([P, CT, DIM], DT.bfloat16, name="w2_a", tag="w2_a")
        nc.sync.dma_start(w1_a[:], w1_d[:, :].rearrange("(c p) f -> p c f", c=CT))
        nc.sync.dma_start(w2_a[:], w2_d[:, :].rearrange("(c p) f -> p c f", c=CT))

        def copy_dve(dst, src):
            nc.vector.tensor_copy(dst, src)

        def copy_act(dst, src):
            nc.scalar.copy(dst, src)

        def kq_proj(pool, wt, srcT, dst, et, jb, eng):
            """dst[:, 512jb:...] = (wt[:,:,128et:...]^T srcT)[:, 512jb:...]."""
            ps = pool.tile([P, IBS], DT.float32, name="kqps", tag="projps")
            for c in range(CT):
                nc.tensor.matmul(
                    ps[:], wt[:, c, P * et:P * (et + 1)],
                    srcT[:, c, IBS * jb:IBS * (jb + 1)],
                    start=(c == 0), stop=(c == CT - 1))
            eng(dst[:, IBS * jb:IBS * (jb + 1)], ps[:])

        # ---- phase P: K (both halves), V, Q-et0.  Copies alternate DVE/ACT
        # ('copy' lives in every act table set, so no table churn).
        engs = [copy_dve, copy_act]
        with tc.tile_pool(name="proj_ps", bufs=2, space="PSUM") as proj_ps, \
             tc.tile_pool(name="v_ps", bufs=2, space="PSUM") as v_ps:

            def v_proj(j, eng):
                ps = v_ps.tile([P, E], DT.float32, name="vps", tag="vps")
                for c in range(CT):
                    nc.tensor.matmul(
                        ps[:], ctx_a[:, c, P * j:P * (j + 1)], wv_a[:, c, :],
                        start=(c == 0), stop=(c == CT - 1))
                v_re = v_t[j].rearrange("p (h x) -> p h x", h=NH)
                eng(v_re[:, :, 0:HD], ps.rearrange("p (h x) -> p h x", h=NH))
                nc.vector.memset(v_re[:, :, HD:65], 1.0)

            for et in range(2):
                for jb in range(IB):
                    kq_proj(proj_ps, wk_a, ctx_a, kT_t[et], et, jb, engs[jb % 2])
            for j in range(JT):
                v_proj(j, engs[j % 2])
            for jb in range(IB):
                kq_proj(proj_ps, wq_a, x_a, qT_t[0], 0, jb, engs[jb % 2])

        # ---- phase A: attention, two heads (one qT/kT tile) at a time ----
        # pr-outer; Q-et1 projection interleaved into pr=0's PE idle;
        # attn@V emission skewed one j behind scores so the next i-block's
        # scores never sit behind a blocked accumulate in the PE FIFO.
        # a2a staging DMAs (Pool queue) fire per (pr=1, ib) chunk.
        def attn_block(pr, ib, acc_pool, s_ps_pool, p_pool, nrm_pool):
            isl = slice(IBS * ib, IBS * (ib + 1))
            accs = [acc_pool.tile([P, IBS], DT.float32, name=f"acc{hh}", tag=f"acc{hh}")
                    for hh in range(2)]
            pts = {}
            for j in range(JT):
                sps = s_ps_pool.tile([P, 2 * IBS], DT.float32, name="sps", tag="sps")
                for hh in range(2):
                    d = slice(HD * hh, HD * (hh + 1))
                    nc.tensor.matmul(
                        sps[:, IBS * hh:IBS * (hh + 1)],
                        kT_t[pr][d, P * j:P * (j + 1)], qT_t[pr][d, isl],
                        start=True, stop=True)
                pt = p_pool.tile([P, 2 * IBS], DT.bfloat16, name="pt", tag="pt")
                nc.scalar.activation(pt[:], sps[:],
                                     mybir.ActivationFunctionType.Exp,
                                     scale=float(HD) ** -0.5)
                pts[j] = pt

                def av(jj):
                    ptj = pts.pop(jj)
                    for hh in range(2):
                        h = 2 * pr + hh
                        nc.tensor.matmul(
                            accs[hh][0:65, :], v_t[jj][:, 65 * h:65 * h + 65],
                            ptj[:, IBS * hh:IBS * (hh + 1)],
                            start=(jj == 0), stop=(jj == JT - 1))

                if j >= 1:
                    av(j - 1)
            av(JT - 1)
            for hh in range(2):
                rcp = nrm_pool.tile([1, IBS], DT.float32, name="rcp", tag="rcp")
                nc.vector.reciprocal(rcp[:], accs[hh][64:65, :])
                bc = nrm_pool.tile([HD, IBS], DT.float32, name="bc", tag="bc")
                nc.gpsimd.partition_broadcast(bc[:], rcp[:])
                nc.vector.tensor_tensor(
                    aT_t[pr][HD * hh:HD * (hh + 1), isl],
                    accs[hh][0:HD, :], bc[:], mybir.AluOpType.mult)

        # phase C uses two half-collectives: the pr=0 half (heads 0,1 of every
        # chunk) ships as soon as pr=0 finishes and overlaps all of pr=1; only
        # the pr=1 half's transfer is exposed at the end.  Each [8P, RQ]
        # buffer block d carries this core's rows-chunk (d%4) for peer d; the
        # batch-duplication (d and d+4) keeps the 8-core AllToAll legal
        # (4-core groups are unsupported), receivers read their batch's half.
        def stage(pr, ib, buf):
            src = aT_t[pr][:, RQ * ib:RQ * (ib + 1)]
            _seng = nc.gpsimd if STAGE_POOL else nc.sync
            _seng.dma_start(buf[P * ib:P * (ib + 1), :], src)
            _seng.dma_start(buf[P * (ib + 4):P * (ib + 5), :], src)

        def fire(in_buf, out_buf):
            if a2a_local:
                nc.sync.dma_start(out_buf[:, :], in_buf[:, :])
            else:
                nc.gpsimd.collective_compute(
                    "AllToAll", mybir.AluOpType.bypass,
                    replica_groups=[[0, 1, 2, 3, 4, 5, 6, 7]],
                    ins=[in_buf.opt()], outs=[out_buf.opt()])

        (a2a_inA, a2a_outA), (a2a_inB, a2a_outB) = a2a_bufs

        # aTf tile et = 2s + half: feature half `half` of peer s's head-group.
        mlp = stack.enter_context(tc.tile_pool(name="mlp", bufs=1))
        aTf_t = [mlp.tile([P, RQ], DT.bfloat16, name=f"aTf{c}", tag=f"aTf{c}") for c in range(CT)]
        hT_t = [mlp.tile([P, RQ], DT.bfloat16, name=f"hT{c}", tag=f"hT{c}") for c in range(CT)]

        def read_a2a(half, out_buf, eng):
            def reads(base):
                for s in range(4):
                    eng.dma_start(aTf_t[2 * s + half][:],
                                  out_buf[P * (s + base):P * (s + base + 1), :])
            if a2a_local:
                reads(0)
            else:
                pid = nc.sync.partition_id()
                with tc.If(pid < 4) as cmp:
                    reads(0)
                with cmp.Else():
                    reads(4)

        with tc.tile_pool(name="p_sb", bufs=3) as p_pool, \
             tc.tile_pool(name="nrm", bufs=4) as nrm_pool, \
             tc.tile_pool(name="s_ps", bufs=2, space="PSUM") as s_ps_pool:
            with tc.tile_pool(name="proj_ps2", bufs=2, space="PSUM") as proj_ps2, \
                 tc.tile_pool(name="acc_psA", bufs=1, space="PSUM") as acc_pool:
                for ib in range(IB):
                    # leftover projection: one Q-et1 jb-chunk per ib (DVE copy)
                    kq_proj(proj_ps2, wq_a, x_a, qT_t[1], 1, ib, copy_dve)
                    attn_block(0, ib, acc_pool, s_ps_pool, p_pool, nrm_pool)
                    stage(0, ib, a2a_inA)
            if SPLIT_A2A:
                fire(a2a_inA, a2a_outA)
                read_a2a(0, a2a_outA, nc.sync)    # lands during pr=1
            with tc.tile_pool(name="acc_psB", bufs=2, space="PSUM") as acc_pool:
                for ib in range(IB):
                    attn_block(1, ib, acc_pool, s_ps_pool, p_pool, nrm_pool)
                    stage(1, ib, a2a_inB)
            if SPLIT_A2A:
                fire(a2a_inB, a2a_outB)
                read_a2a(1, a2a_outB, nc.scalar if READ_ACT else nc.sync)
            else:
                fire(a2a_inA, a2a_outA)
                fire(a2a_inB, a2a_outB)
                read_a2a(0, a2a_outA, nc.sync)
                read_a2a(1, a2a_outB, nc.scalar if READ_ACT else nc.sync)

        # ---- phase M: Y^T = W1^T A^T (relu) ; Z = H W2 ; LayerNorm ----
        # All 8 ft accumulators live in PSUM at once (8 banks) so ALL matmuls
        # against A-half tiles (even et, resident before the B collective
        # lands) run first, overlapping the exposed B transfer.
        with tc.tile_pool(name="y_ps", bufs=1, space="PSUM") as y_ps_pool:
            yps = [y_ps_pool.tile([P, RQ], DT.float32, name=f"yps{ft}", tag=f"yps{ft}")
                   for ft in range(FT)]
            for ei, et in enumerate([0, 2, 4, 6]):      # A-half: resident early
                for ft in range(FT):
                    nc.tensor.matmul(
                        yps[ft][:], w1_a[:, et, P * ft:P * (ft + 1)], aTf_t[et][:],
                        start=(ei == 0), stop=False)
            for ft in range(FT):                         # B-half; relu pipelined
                for ei, et in enumerate([1, 3, 5, 7]):
                    nc.tensor.matmul(
                        yps[ft][:], w1_a[:, et, P * ft:P * (ft + 1)], aTf_t[et][:],
                        start=False, stop=(ei == 3))
                nc.vector.tensor_scalar_max(hT_t[ft][:], yps[ft][:], 0.0)
        with tc.tile_pool(name="z_ps", bufs=3, space="PSUM") as z_ps_pool, \
             tc.tile_pool(name="ln", bufs=2) as ln_pool:
            for it in range(RQ // P):
                zps = z_ps_pool.tile([P, DIM], DT.float32, name="zps", tag="zps")
                for gt in range(2):
                    for ft in range(FT):
                        nc.tensor.matmul(
                            zps[:, IBS * gt:IBS * (gt + 1)],
                            hT_t[ft][:, P * it:P * (it + 1)],
                            w2_a[:, ft, IBS * gt:IBS * (gt + 1)],
                            start=(ft == 0), stop=(ft == FT - 1))
                stats = ln_pool.tile([P, 2, 6], DT.float32, name="stats", tag="stats")
                for sg in range(2):
                    nc.vector.bn_stats(stats[:, sg, :], zps[:, IBS * sg:IBS * (sg + 1)])
                mv = ln_pool.tile([P, 2], DT.float32, name="mv", tag="mv")
                nc.vector.bn_aggr(mv[:], stats[:])
                # mv[:,1] := 1/sqrt(var + eps).  One act-table swap (exp ->
                # sqrt set) at it=0, hidden under the remaining MLP2 matmuls.
                nc.scalar.activation(mv[:, 1:2], mv[:, 1:2],
                                     mybir.ActivationFunctionType.Sqrt,
                                     bias=eps_t[:])
                nc.vector.reciprocal(mv[:, 1:2], mv[:, 1:2])
                zn = ln_pool.tile([P, DIM], DT.float32, name="zn", tag="zn")
                nc.vector.tensor_scalar(
                    out=zn[:], in0=zps[:], scalar1=mv[:, 0:1], scalar2=mv[:, 1:2],
                    op0=mybir.AluOpType.subtract, op1=mybir.AluOpType.mult)
                ot = ln_pool.tile([P, DIM], DT.float32, name="ot", tag="ot")
                nc.gpsimd.tensor_tensor(ot[:], zn[:], gamma_bc[:], mybir.AluOpType.mult)
                nc.sync.dma_start(out_d[P * it:P * (it + 1), :], ot[:])


def _build(k_rep, a2a_local=False):
    nc = bacc.Bacc(None, num_devices=8)

    xT_d = nc.dram_tensor("xT", [DIM, N], DT.bfloat16, kind="ExternalInput")
    ctxT_d = nc.dram_tensor("ctxT", [DIM, N], DT.bfloat16, kind="ExternalInput")
    wq_d = nc.dram_tensor("wq", [DIM, E], DT.bfloat16, kind="ExternalInput")
    wk_d = nc.dram_tensor("wk", [DIM, E], DT.bfloat16, kind="ExternalInput")
    wv_d = nc.dram_tensor("wv", [DIM, E], DT.bfloat16, kind="ExternalInput")
    w1_d = nc.dram_tensor("w1", [DIM, DIM], DT.bfloat16, kind="ExternalInput")
    w2_d = nc.dram_tensor("w2", [DIM, DIM], DT.bfloat16, kind="ExternalInput")
    gamma_d = nc.dram_tensor("gamma", [1, DIM], DT.float32, kind="ExternalInput")
    out_d = nc.dram_tensor("out", [RQ, DIM], DT.float32, kind="ExternalOutput")
    tensors = (xT_d, ctxT_d, wq_d, wk_d, wv_d, w1_d, w2_d, gamma_d, out_d)

    with tile.TileContext(nc) as tc:
        nc.gpsimd.load_library(library_config.attnmlp)
        frees = []
        a2a_bufs = []
        for nm in ("A", "B"):
            ib_t, ib_free = tc.tile([8 * P, RQ], DT.bfloat16, space="DRAM",
                                    name=f"a2a_in{nm}")
            ob_t, ob_free = tc.tile([8 * P, RQ], DT.bfloat16, space="DRAM",
                                    addr_space="Shared", name=f"a2a_out{nm}")
            a2a_bufs.append((ib_t, ob_t))
            frees += [ib_free, ob_free]
        for _ in range(k_rep):
            _emit_body(nc, tc, tensors, a2a_bufs, a2a_local=a2a_local)
        for f in frees:
            f()

    nc.finalize()
    return nc


def build_program():
    return _build(1)


def build_program_k(k_rep):
    return _build(k_rep)


def _get_program():
    global _PROGRAM
    if _PROGRAM is None:
        _PROGRAM = build_program()
    return _PROGRAM


def prepare_in_maps(x, context, w_kv, w_q, w_out1, w_out2, gamma):
    x = np.asarray(x, np.float32)
    context = np.asarray(context, np.float32)
    w_kv = np.asarray(w_kv, np.float32)
    w_q = np.asarray(w_q, np.float32)
    w1 = np.ascontiguousarray(np.asarray(w_out1, np.float32).astype(BF16))
    w2 = np.ascontiguousarray(np.asarray(w_out2, np.float32).astype(BF16))
    gamma = np.asarray(gamma, np.float32).reshape(1, DIM)
    xT = [np.ascontiguousarray(x[b].T.astype(BF16)) for b in range(2)]
    ctxT = [np.ascontiguousarray(context[b].T.astype(BF16)) for b in range(2)]
    in_maps = []
    for c in range(8):
        b, g = divmod(c, 4)
        e0 = E * g
        in_maps.append({
            "xT": xT[b],
            "ctxT": ctxT[b],
            "wq": np.ascontiguousarray(w_q[:, e0:e0 + E].astype(BF16)),
            "wk": np.ascontiguousarray(w_kv[:, e0:e0 + E].astype(BF16)),
            "wv": np.ascontiguousarray(w_kv[:, DIM + e0:DIM + e0 + E].astype(BF16)),
            "w1": w1,
            "w2": w2,
            "gamma": gamma,
        })
    return in_maps


def assemble_output(per_core_outs):
    out = np.empty((2, N, DIM), np.float32)
    for c in range(8):
        b, g = divmod(c, 4)
        out[b, RQ * g:RQ * (g + 1), :] = per_core_outs[c]
    return out


def kernel(x, context, w_kv, w_q, w_out1, w_out2, gamma):
    global LAST_RUN
    in_maps = prepare_in_maps(x, context, w_kv, w_q, w_out1, w_out2, gamma)
    nc = _get_program()
    res = run_bass_kernel_spmd(nc, in_maps, list(range(8)))
    LAST_RUN = res
    return assemble_output([res.results[c]["out"] for c in range(8)])
